# revision 9
# baseline (speedup 1.0000x reference)
"""Multi-head attention (RoPE) Trainium2 kernel, 8-way sharded.

Sharding: core c handles batch b = c//4 and 4 heads h0 = 4*(c%4).
Per-core device program (all layouts chosen so no on-device transposes
are needed; host pre-transposes/slices the inputs):

  inputs (per core):
    xT   [1024, 2048]  = x[b].T
    wqkT [1024, 512]   = concat(w_q_rows, w_k_rows).T  (4 heads' q/k rows)
    wvT  [1024, 256]   = w_v_rows.T
    woT  [256, 1024]   = w_out[:, head_cols].T
    cosT [128, 2048]   rope cos table, row r -> dim r%64 (2 heads stacked)
    sinT [128, 2048]   rope sin table with rotate-half sign baked in

  program:
    qkT  = (wqkT.T @ xT tiles) + rope        [512, 2048]  (feat-major)
    V'   = x @ w_v.T  (+ ones col per head)  [2048, 4*65] (token-major)
    per head: S.T[kt,qt] = k'T.T @ q'T ; P = exp(0.125*S) (no max; |S|<9)
              outT[65,qt] = [V|1].T @ P  (row 64 = softmax denominator)
              attn_outT = outT[:64] * bcast(1/outT[64])
    y = attn_out @ w_out_slice.T             [2048, 1024] (partial sum)

  host: y[b] = sum of the 4 per-core partials.
"""

import numpy as np

B = 2
N = 2048
C = 1024
H_TOT = 16
HD = 64
HC = 4  # heads per core
N_CORES = 8
ROPE_BASE = 10000.0

_PROGRAM = None


def _rope_tables():
    inv_freq = 1.0 / (ROPE_BASE ** (np.arange(0, HD, 2, dtype=np.float32) / HD))
    t = np.arange(N, dtype=np.float32)
    freqs = np.einsum("i,j->ij", t, inv_freq).astype(np.float32)  # [N, 32]
    emb = np.concatenate([freqs, freqs], axis=-1)  # [N, 64]
    cos = np.cos(emb).astype(np.float32)
    sin = np.sin(emb).astype(np.float32)
    cosT = np.ascontiguousarray(np.tile(cos.T, (2, 1)))  # [128, 2048]
    sinT = sin.T.copy()  # [64, 2048]
    sinT_signed = np.concatenate([-sinT[:32], sinT[32:]], axis=0)  # sign for rot-half
    sinT2 = np.ascontiguousarray(np.tile(sinT_signed, (2, 1)))  # [128, 2048]
    return cosT, sinT2


def _build_program(debug=False):
    import concourse.mybir as mybir
    import concourse.tile as tile
    from concourse import bacc

    f32 = mybir.dt.float32
    f32r = mybir.dt.float32r
    MUL = mybir.AluOpType.mult
    ADD = mybir.AluOpType.add
    EXP = mybir.ActivationFunctionType.Exp

    nc = bacc.Bacc("TRN2", target_bir_lowering=False, debug=False, num_devices=N_CORES)

    xT_d = nc.dram_tensor("xT", [C, N], f32r, kind="ExternalInput").ap()
    wqk_d = nc.dram_tensor("wqkT", [C, 2 * HC * HD], f32r, kind="ExternalInput").ap()
    wv_d = nc.dram_tensor("wvT", [C, HC * HD], f32r, kind="ExternalInput").ap()
    wo_d = nc.dram_tensor("woT", [HC * HD, C], f32r, kind="ExternalInput").ap()
    cos_d = nc.dram_tensor("cosT", [128, N], f32, kind="ExternalInput").ap()
    sin_d = nc.dram_tensor("sinT", [128, N], f32, kind="ExternalInput").ap()
    y_d = nc.dram_tensor("y", [N, C], f32, kind="ExternalOutput").ap()
    if debug:
        qk_dbg = nc.dram_tensor("qk_dbg", [4, 128, N], f32, kind="ExternalOutput").ap()
        vv_dbg = nc.dram_tensor("vv_dbg", [128, 16 * HC * (HD + 1)], f32, kind="ExternalOutput").ap()
        es_dbg = nc.dram_tensor("es_dbg", [128, N], f32, kind="ExternalOutput").ap()
        nb_dbg = nc.dram_tensor("nb_dbg", [HD + 1, N], f32, kind="ExternalOutput").ap()
        ao_dbg = nc.dram_tensor("ao_dbg", [2, 128, N], f32, kind="ExternalOutput").ap()


    with tile.TileContext(nc) as tc:
        with (
            tc.tile_pool(name="persist", bufs=1) as persist,
            tc.tile_pool(name="psum", bufs=2, space="PSUM") as psp,
        ):
            # live through qkv + attention phases
            qk = [
                persist.tile([128, N], f32r, tag=f"qk{i}", name=f"qk{i}")
                for i in range(4)
            ]
            vv = persist.tile([128, 16, HC, HD + 1], f32r, tag="vv", name="vv")

            with tc.tile_pool(name="phaseA", bufs=1) as pa:
                xT = pa.tile([128, 8, N], f32r, tag="xT", name="xT")
                wqk = pa.tile([128, 8, 2 * HC * HD], f32r, tag="wqk", name="wqk")
                wv = pa.tile([128, 8, HC * HD], f32r, tag="wv", name="wv")
                cosT = pa.tile([128, N], f32, tag="cosT")
                sinT = pa.tile([128, N], f32, tag="sinT")
                nc.sync.dma_start(cosT[:], cos_d[:, :])
                nc.sync.dma_start(sinT[:], sin_d[:, :])
                for i in range(8):
                    nc.sync.dma_start(xT[:, i, :], xT_d[i * 128 : (i + 1) * 128, :])
                    nc.sync.dma_start(wqk[:, i, :], wqk_d[i * 128 : (i + 1) * 128, :])
                    nc.sync.dma_start(wv[:, i, :], wv_d[i * 128 : (i + 1) * 128, :])

                # --- q/k projection + rope (feat-major: [feat, tok]) ---
                for pt in range(4):
                    bp = psp.tile([128, N], f32, tag="big", name=f"qkps{pt}")
                    for tck in range(4):
                        sl = slice(tck * 512, (tck + 1) * 512)
                        for ct in range(8):
                            nc.tensor.matmul(
                                bp[:, sl],
                                wqk[:, ct, pt * 128 : (pt + 1) * 128],
                                xT[:, ct, sl],
                                start=(ct == 0),
                                stop=(ct == 7),
                            )
                    for tck in range(4):
                        sl = slice(tck * 512, (tck + 1) * 512)
                        t_sb = pa.tile([128, 512], f32, tag="ropet", bufs=2, name="rt")
                        u_sb = pa.tile([128, 512], f32, tag="ropeu", bufs=2, name="ru")
                        nc.vector.tensor_tensor(t_sb[:], bp[:, sl], cosT[:, sl], MUL)
                        for o_lo, i_lo in [(0, 32), (32, 0), (64, 96), (96, 64)]:
                            nc.vector.tensor_tensor(
                                u_sb[o_lo : o_lo + 32, :],
                                bp[i_lo : i_lo + 32, sl],
                                sinT[o_lo : o_lo + 32, sl],
                                MUL,
                            )
                        nc.vector.tensor_tensor(qk[pt][:, sl], t_sb[:], u_sb[:], ADD)

                # --- v projection (token-major: [tok, feat]) + ones column ---
                for tt in range(16):
                    nc.vector.memset(vv[:, tt, :, HD].bitcast(f32), 1.0)
                    vp = psp.tile([128, HC * HD], f32, tag="big", name=f"vps{tt}")
                    for ct in range(8):
                        nc.tensor.matmul(
                            vp[:, :],
                            xT[:, ct, tt * 128 : (tt + 1) * 128],
                            wv[:, ct, :],
                            start=(ct == 0),
                            stop=(ct == 7),
                        )
                    nc.vector.tensor_copy(
                        vv[:, tt, :, 0:HD],
                        vp[:].rearrange("p (h d) -> p h d", h=HC),
                    )

                if debug:
                    for pt in range(4):
                        nc.sync.dma_start(qk_dbg[pt], qk[pt][:].bitcast(f32))
                    nc.sync.dma_start(vv_dbg[:, :], vv[:].rearrange("p a b c -> p (a b c)").bitcast(f32))

            with tc.tile_pool(name="phaseBC", bufs=1) as pbc:
                ao = [
                    pbc.tile([128, N], f32r, tag=f"ao{i}", name=f"ao{i}")
                    for i in range(2)
                ]
                wo = pbc.tile([128, 2, C], f32r, tag="wo", name="wo")
                for i in range(2):
                    nc.sync.dma_start(wo[:, i, :], wo_d[i * 128 : (i + 1) * 128, :])

                # --- attention, head by head ---
                for h in range(HC):
                    qpt = h // 2
                    roff = (h % 2) * 64
                    pv = psp.tile([HD + 1, N], f32, tag="big", name=f"pv{h}")
                    for kt in range(16):
                        sp = psp.tile([128, N], f32, tag="big", name=f"sps{h}_{kt}")
                        for qc in range(4):
                            sl = slice(qc * 512, (qc + 1) * 512)
                            nc.tensor.matmul(
                                sp[:, sl],
                                
                                    qk[2 + qpt][
                                        roff : roff + 64, kt * 128 : (kt + 1) * 128
                                    ]
                                ,
                                qk[qpt][roff : roff + 64, sl],
                                start=True,
                                stop=True,
                            )
                        es = pbc.tile([128, N], f32r, tag="es", bufs=2, name="es")
                        nc.scalar.activation(es[:], sp[:], EXP, scale=float(HD**-0.5))
                        if debug and h == 0 and kt == 0:
                            nc.sync.dma_start(es_dbg[:, :], es[:].bitcast(f32))
                        for qc in range(4):
                            sl = slice(qc * 512, (qc + 1) * 512)
                            nc.tensor.matmul(
                                pv[:, sl],
                                vv[:, kt, h, :],
                                es[:, sl],
                                start=(kt == 0),
                                stop=(kt == 15),
                            )
                    rb = pbc.tile([1, N], f32, tag="rb", bufs=2, name="rb")
                    nb = pbc.tile([HD, N], f32, tag="nb", bufs=2, name="nb")
                    nc.vector.reciprocal(rb[0:1, :], pv[HD : HD + 1, :])
                    nc.gpsimd.partition_broadcast(nb[0:HD, :], rb[0:1, :])
                    nc.vector.tensor_tensor(
                        ao[qpt][roff : roff + 64, :], pv[0:HD, :], nb[0:HD, :], MUL
                    )
                    if debug and h == 0:
                        nc.sync.dma_start(nb_dbg[0:HD, :], nb[:])
                        nc.sync.dma_start(nb_dbg[HD : HD + 1, :], rb[:])

                if debug:
                    for i in range(2):
                        nc.sync.dma_start(ao_dbg[i], ao[i][:].bitcast(f32))

                # --- output projection: y[tok, out_ch] (partial, this core's heads) ---
                for tt in range(16):
                    yps = psp.tile([128, C], f32, tag="big", name=f"yps{tt}")
                    for oc in range(2):
                        osl = slice(oc * 512, (oc + 1) * 512)
                        for ft in range(2):
                            nc.tensor.matmul(
                                yps[:, osl],
                                ao[ft][:, tt * 128 : (tt + 1) * 128],
                                wo[:, ft, osl],
                                start=(ft == 0),
                                stop=(ft == 1),
                            )
                    ysb = pbc.tile([128, C], f32, tag="y", bufs=3, name="ysb")
                    nc.vector.tensor_copy(ysb[:], yps[:])
                    nc.sync.dma_start(y_d[tt * 128 : (tt + 1) * 128, :], ysb[:])

    nc.compile()
    return nc


def _get_program():
    global _PROGRAM
    if _PROGRAM is None:
        _PROGRAM = _build_program()
    return _PROGRAM


def _make_in_maps(x, w_qkv, w_out):
    x = np.asarray(x, dtype=np.float32)
    w_qkv = np.asarray(w_qkv, dtype=np.float32)
    w_out = np.asarray(w_out, dtype=np.float32)
    cosT, sinT = _rope_tables()
    in_maps = []
    for c in range(N_CORES):
        b = c // 4
        h0 = HC * (c % 4)
        rows = np.arange(h0 * HD, (h0 + HC) * HD)
        wq = w_qkv[rows]  # [256, 1024]
        wk = w_qkv[C + rows]
        wv = w_qkv[2 * C + rows]
        in_maps.append(
            {
                "xT": np.ascontiguousarray(x[b].T),
                "wqkT": np.ascontiguousarray(np.concatenate([wq, wk], 0).T),
                "wvT": np.ascontiguousarray(wv.T),
                "woT": np.ascontiguousarray(w_out[:, rows].T),
                "cosT": cosT,
                "sinT": sinT,
            }
        )
    return in_maps


def run(inputs, trace=False, trace_cores=None):
    from concourse.bass_utils import run_bass_kernel_spmd

    nc = _get_program()
    in_maps = _make_in_maps(inputs["x"], inputs["w_qkv"], inputs["w_out"])
    res = run_bass_kernel_spmd(
        nc,
        in_maps,
        core_ids=list(range(N_CORES)),
        trace=trace,
        trace_cores=trace_cores,
    )
    y = np.zeros((B, N, C), dtype=np.float32)
    for c in range(N_CORES):
        y[c // 4] += res.results[c]["y"]
    return y, res


def kernel(**inputs) -> np.ndarray:
    y, _ = run(inputs, trace=False)
    return y
